# revision 27
# baseline (speedup 1.0000x reference)
"""Bass/Trainium2 kernel for nn_DFTLayer: out[b,f,k] = DFT_1024(x[b,f,:]).

reference: real = einsum('bfs,ks->bfk', x, wcos); imag = ... wsin
           out  = complex(real, -imag) = FFT(x),  x: [16, 1024, 1024] f32.

Strategy (8 NeuronCores, data-parallel over batch, 2 batches/core):
  - Hermitian symmetry (x real): out[k] = conj(out[N-k]); the device only
    computes freq cols k = 1..512, col 0 is a host row-sum, cols 513..1023
    are a host conjugate mirror.
  - Two fold levels (radix-2 DIF steps) done host-side while sharding:
      u[s] = x[s]+x[N-s], v[s] = x[s]-x[N-s]          (length 512)
      ue/uo = u[s] +/- u[512-s], ve/vo = v[s] +/- v[512-s]  (length 256)
    leaving four independent 256-long contractions per row:
      re_even[k=2m]  = ue . cos(2pi m s/512)   (+ edge terms, host)
      re_odd[k=2m+1] = uo . cos(2pi(2m+1)s/1024)
      im_even        = vo . -sin(2pi m s/512)
      im_odd         = ve . -sin(2pi(2m+1)s/1024)  (+ edge, host)
  - All device I/O is fp16 (gate is rel_fro < 2e-2; measured ~3e-4):
    input 4.2 MB + weights 0.5 MB + output 4.2 MB per core, vs 19 MB at
    fp32 without the second fold -- the kernel is HBM-bound.
  - The device is a pure streaming GEMM: weights stationary [128,128],
    moving operand = pre-transposed fold outputs [s=128, f=512] fp16,
    PSUM accumulates the two s-chunks, ACT+DVE copy PSUM->SBUF fp16.
    64 matmuls of N=512 total. No on-device transposes or folds.
  - Host assembles: parity interleave, fold edge terms, k=0 column,
    Hermitian mirror.
"""

import sys

for _p in ("/opt/trn_rl_repo", "/root/.axon_site/_ro/trn_rl_repo"):
    if _p not in sys.path:
        sys.path.append(_p)

import numpy as np
from contextlib import ExitStack

N_CORES = 8
B, F_FULL, S = 16, 1024, 1024          # x: [B, F_FULL, S]
F = (B // N_CORES) * F_FULL            # 2048 rows per core
NB = 4                                 # f-blocks per core (of 512 rows)
FB = F // NB                           # 512 rows per f-block
NMAT = 4                               # ue, uo, vo, ve

_CACHE = {}


def _weights():
    """The four 256x256 fold kernels, packed [128, 2048] fp16 in tile order
    t = 4*M + 2*i + jc  (M: matrix, i: s-chunk, jc: m-chunk)."""
    s = np.arange(256)[:, None].astype(np.float64)
    m = np.arange(256)[None, :].astype(np.float64)
    mats = [
        np.cos(2 * np.pi * (m + 1) * s / 512),        # WE_RE, k = 2(m+1)
        np.cos(2 * np.pi * (2 * m + 1) * s / 1024),   # WO_RE, k = 2m+1
        -np.sin(2 * np.pi * (m + 1) * s / 512),       # WE_IM
        -np.sin(2 * np.pi * (2 * m + 1) * s / 1024),  # WO_IM
    ]
    w = np.empty((128, 16 * 128), np.float16)
    for M, W in enumerate(mats):
        Wf = W.astype(np.float32)
        for i in range(2):
            for jc in range(2):
                t = 4 * M + 2 * i + jc
                w[:, t * 128:(t + 1) * 128] = Wf[
                    i * 128:(i + 1) * 128, jc * 128:(jc + 1) * 128]
    return w


def _build():
    """Build + compile the per-core Bass program (cached)."""
    if "nc" in _CACHE:
        return _CACHE["nc"]

    from concourse import bacc, tile, mybir

    f32 = mybir.dt.float32
    f16 = mybir.dt.float16

    nc = bacc.Bacc("TRN2", target_bir_lowering=False, debug=False)

    pt_d = nc.dram_tensor("pt", [2 * NMAT, 128, NB * FB], f16, kind="ExternalInput")
    w_d = nc.dram_tensor("w", [128, 16 * 128], f16, kind="ExternalInput")
    o_d = nc.dram_tensor("o", [2 * NMAT, 128, NB, FB], f16, kind="ExternalOutput")

    with tile.TileContext(nc) as tc, ExitStack() as ctx:
        wpool = ctx.enter_context(tc.tile_pool(name="w", bufs=1))
        pspool = ctx.enter_context(tc.tile_pool(name="ps", bufs=4, space="PSUM"))
        opool = ctx.enter_context(tc.tile_pool(name="o", bufs=3))

        # 512 KB moving-operand DMAs round-robined over all three rings
        # (sync/scalar HWDGE + gpsimd SWDGE) so each ring's per-DMA
        # completion gap overlaps the other rings' transfers. The DFT
        # kernels are split in two so the first matmul only waits on the
        # M=0/1 half.
        # DFT-kernel halves lead the two HWDGE rings so the first matmul
        # can start as soon as c0/c1 land right behind them. Even chunks
        # on sync, odd on scalar; both rings drain in consumption order.
        # c0/c1 are split in half for an earlier first matmul.
        w01_t = wpool.tile([128, 8 * 128], f16, tag="w01")
        nc.sync.dma_start(w01_t[:], w_d[:, 0:1024])
        w23_t = wpool.tile([128, 8 * 128], f16, tag="w23")
        nc.scalar.dma_start(w23_t[:], w_d[:, 1024:2048])
        w_ts = [w01_t, w23_t]

        # PE warm-up: the HAM throttle runs the PE at 1.2 GHz for its
        # first ~3.4 us of activity. Burn that window on dummy matmuls
        # over an iota tile while the input DMAs stream, so the real
        # matmuls start at full clock.
        warm = wpool.tile([128, 128], mybir.dt.int16, tag="warm")
        nc.gpsimd.iota(warm[:], pattern=[[1, 128]], base=0, channel_multiplier=0)
        wf = warm[:].bitcast(f16)
        ps_warm = pspool.tile([128, 2, FB], f32, name="ps_warm", tag="ps")
        for _ in range(40):
            nc.tensor.matmul(ps_warm[:, 0, 0:128], wf, wf, start=True, stop=True)

        mv_tiles = {}

        def mv(c, fb):
            t, base = mv_tiles[c][0] if fb < 2 else mv_tiles[c][-1]
            return t[:, (fb - base) * FB:(fb - base + 1) * FB]

        for c in range(2 * NMAT):
            eng = nc.sync if c % 2 == 0 else nc.scalar
            if c < 2:
                ta = wpool.tile([128, 2 * FB], f16, tag=f"pt{c}a")
                eng.dma_start(ta[:], pt_d[c, :, 0:2 * FB])
                tb = wpool.tile([128, 2 * FB], f16, tag=f"pt{c}b")
                eng.dma_start(tb[:], pt_d[c, :, 2 * FB:4 * FB])
                mv_tiles[c] = [(ta, 0), (tb, 2)]
            else:
                t = wpool.tile([128, NB * FB], f16, tag=f"pt{c}")
                eng.dma_start(t[:], pt_d[c, :, :])
                mv_tiles[c] = [(t, 0)]

        NBLK = 2 * NMAT
        for M in range(NMAT):
            for jc in range(2):
                blk = 2 * M + jc
                fine = blk >= NBLK - 2
                wt = w_ts[M // 2]
                ob = opool.tile([128, NB, FB], f16)
                # both PSUM halves of the block live together so each
                # stationary (i) serves 4 consecutive matmuls (1 LDWEIGHTS)
                ph = [pspool.tile([128, 2, FB], f32, name=f"ps{blk}_{h}", tag="ps")
                      for h in range(2)]
                for i in range(2):
                    col = ((M % 2) * 4 + 2 * i + jc) * 128
                    for fb in range(NB):
                        nc.tensor.matmul(
                            ph[fb // 2][:, fb % 2, :],
                            wt[:, col:col + 128],
                            mv(2 * M + i, fb),
                            start=(i == 0),
                            stop=(i == 1),
                        )
                for h in range(2):
                    cp = nc.scalar.copy if h == 0 else nc.vector.tensor_copy
                    cp(ob[:, 2 * h:2 * h + 2, :], ph[h][:])
                    if fine:
                        # tail blocks: store each half on an HWDGE ring
                        # (idle by now) so the last bytes land early
                        eng = nc.sync if (blk + h) % 2 == 0 else nc.scalar
                        eng.dma_start(o_d[blk, :, 2 * h:2 * h + 2, :],
                                      ob[:, 2 * h:2 * h + 2, :])
                if not fine:
                    nc.gpsimd.dma_start(o_d[blk, :, :, :], ob[:])

    nc.compile()
    _CACHE["nc"] = nc
    return nc


def kernel(x, wsin, wcos):
    from concourse.bass_utils import run_bass_kernel_spmd

    x = np.asarray(x, dtype=np.float32)

    nc = _build()
    w = _CACHE.setdefault("w", _weights())

    xf = x.reshape(B * F_FULL, S)
    # fold level 1: u[s] = x[s]+x[N-s], v = x[s]-x[N-s]  (s = 1..511)
    a, b = xf[:, 1:512], xf[:, :512:-1]
    u = np.empty((B * F_FULL, 512), np.float32)
    v = np.empty_like(u)
    u[:, 0] = xf[:, 0]
    v[:, 0] = 0.0
    np.add(a, b, out=u[:, 1:])
    np.subtract(a, b, out=v[:, 1:])
    # fold level 2 -> P columns [ue | uo | vo | ve] (matmul operand order)
    P = np.empty((B * F_FULL, 1024), np.float16)
    ua, ub = u[:, 1:256], u[:, :256:-1]
    va, vb = v[:, 1:256], v[:, :256:-1]
    P[:, 0] = u[:, 0]
    P[:, 1:256] = ua + ub                    # ue
    P[:, 256] = u[:, 0]
    P[:, 257:512] = ua - ub                  # uo
    P[:, 512] = 0.0
    P[:, 513:768] = va - vb                  # vo
    P[:, 768] = 0.0
    P[:, 769:1024] = va + vb                 # ve
    # edge terms used host-side
    u256 = u[:, 256].reshape(B, F_FULL)
    v256 = v[:, 256].reshape(B, F_FULL)
    x512 = xf[:, 512].reshape(B, F_FULL)
    rowsum = xf.sum(axis=1, dtype=np.float32).reshape(B, F_FULL)

    # per-core moving-operand layout [c=8, p=128, fb*512+j]:
    # pt[c, p, fb*512+j] = P[fb*512 + j, 128*c + p]
    in_maps = []
    for core in range(N_CORES):
        Pc = P[core * F:(core + 1) * F]
        ptc = np.ascontiguousarray(
            Pc.reshape(NB, FB, 8, 128).transpose(2, 3, 0, 1).reshape(8, 128, NB * FB))
        in_maps.append({"pt": ptc, "w": w})

    res = run_bass_kernel_spmd(
        nc, in_maps, core_ids=list(range(N_CORES)), **_CACHE.get("run_kwargs", {})
    )
    kernel.last_results = res

    out = np.empty((B, F_FULL, S), dtype=np.complex64)
    fv = out.view(np.float32).reshape(B, F_FULL, 2 * S)
    alt_e = np.where(np.arange(1, 257) % 2 == 0, np.float32(1), np.float32(-1))
    alt_m = np.where(np.arange(256) % 2 == 0, np.float32(1), np.float32(-1))
    bpc = B // N_CORES
    for core in range(N_CORES):
        b0 = core * bpc
        # o[blk, p, fb, j]: blk = 2*M + jc, value = Out_M[m=128*jc+p, f=fb*512+j]
        O = res.results[core]["o"].astype(np.float32)
        O = O.reshape(NMAT, 2 * 128, NB * FB)        # [M, m, f]
        O = O.transpose(0, 2, 1).reshape(NMAT, bpc, F_FULL, 256)
        A_e, A_o, B_e, B_o = O[0], O[1], O[2], O[3]
        re_e = A_e + alt_e * u256[b0:b0 + bpc, :, None] + x512[b0:b0 + bpc, :, None]
        re_o = A_o - x512[b0:b0 + bpc, :, None]
        im_e = B_e
        im_o = B_o - alt_m * v256[b0:b0 + bpc, :, None]
        blk = fv[b0:b0 + bpc]
        blk[:, :, 0] = rowsum[b0:b0 + bpc]
        blk[:, :, 1] = 0.0
        # k = 1..512: even k=2(m+1) from *_e, odd k=2m+1 from *_o
        blk[:, :, 2:2 * 512 + 2:4] = re_o            # re, k odd  (1,3,..,511)
        blk[:, :, 4:2 * 512 + 4:4] = re_e            # re, k even (2,4,..,512)
        blk[:, :, 3:2 * 512 + 3:4] = im_o            # im, k odd
        blk[:, :, 5:2 * 512 + 5:4] = im_e            # im, k even
        # Hermitian mirror: out[k] = conj(out[1024-k]) for k = 513..1023
        re = blk[:, :, 2:2 * 512 + 2:2]
        im = blk[:, :, 3:2 * 512 + 3:2]
        blk[:, :, 2 * 512 + 2::2] = re[:, :, 510::-1]
        blk[:, :, 2 * 512 + 3::2] = -im[:, :, 510::-1]
    return out
